# revision 1
# baseline (speedup 1.0000x reference)
import ctypes
import math
import os
import subprocess
import tempfile

import numpy as np
import jax
import jax.numpy as jnp
from jax.sharding import Mesh, PartitionSpec as P

if hasattr(jax, 'shard_map'):  # jax >= 0.8
    _shard_map = jax.shard_map
else:  # pragma: no cover
    from jax.experimental.shard_map import shard_map as _shard_map

# Problem: nn_CGABlock_38087769981516 — data-parallel over 8 NeuronCores.
B, C_IN, C_OUT, V = 512, 64, 64, 25
MID = C_IN // 8
INTER = C_OUT // 2
BN_EPS = 1e-5
N_CORES = 8

_PREC = jax.lax.Precision.DEFAULT
_RSQV = 1.0 / math.sqrt(V)

# Wall-clock over the axon tunnel is dominated by wire bytes (~89MB/s up,
# ~56MB/s down, ~70ms RTT) — device compute hides almost entirely under the
# RTT. So the wire payloads are compressed: x travels as fp16 (adds ~3e-4
# rel-err) and the result travels as int8 per-(sample,channel) quantized
# *delta* (out - x, adds ~2e-3 rel-err; budget is 2e-2) plus fp16 scales.
# The exact fp32 x is added back on host. Params are baked into the jitted
# graph as constants (re-traced only if their bytes change), so the only
# per-call upload is x itself, passed as host numpy straight into the jit —
# measured faster than dispatching on pre-committed sharded buffers.


def _fused(xh, w1, b1, w2, b2, w3, b3, dw, db, edge_w, edge_b, att_w, att_b,
           A_static, alpha, cc1_w, cc1_b, bn_g, bn_b, bn_m, bn_v,
           cc2_w, cc2_b, cs_w, cs_b):
    x = xh.astype(jnp.float32)
    x1 = jnp.matmul(w1[None], x, precision=_PREC) + b1[:, None]
    x2 = jnp.matmul(w2[None], x, precision=_PREC) + b2[:, None]
    x3 = jnp.matmul(w3[None], x, precision=_PREC) + b3[:, None]
    # Grouped pairwise-diff conv, exact rank-1 form. Pairing: group g reads
    # channels (2g, 2g+1) of concat([d1, d2]) — g<4 from d1, g>=4 from d2.
    x1r = x1.reshape(-1, MID // 2, 2, V)
    x2r = x2.reshape(-1, MID // 2, 2, V)
    dwa = dw[:MID // 2].reshape(1, MID // 2, 2, 1)
    dwb = dw[MID // 2:].reshape(1, MID // 2, 2, 1)
    f1 = jnp.concatenate([(x1r * dwa).sum(2), (x2r * dwb).sum(2)], axis=1)
    f2 = jnp.concatenate([(x2r * dwa).sum(2), (x1r * dwb).sum(2)], axis=1)
    A_dyn = jnp.tanh(f1[:, :, :, None] - f2[:, :, None, :]
                     + db[None, :, None, None]).reshape(-1, MID, V * V)
    A_mix = jnp.matmul(edge_w[None], A_dyn, precision=_PREC) \
        + edge_b[None, :, None]
    att = jnp.tanh((x1[:, :, :, None] * x2[:, :, None, :]) * _RSQV) \
        .reshape(-1, MID, V * V)
    att_m = jnp.matmul(att_w[None], att, precision=_PREC) \
        + att_b[None, :, None]
    bs = x3.shape[0]
    # Per-(sample,channel) 1x25 @ 25x25 contractions lower poorly as 8192
    # tiny PE matmuls; elementwise mul + reduce keeps them on DVE instead.
    att_m4 = att_m.reshape(bs, C_OUT, V, V)
    x_att = (att_m4 * x3[:, :, :, None]).sum(2)
    A_out4 = (A_static.reshape(1, 1, V * V)
              + alpha * A_mix).reshape(bs, C_OUT, V, V)
    x_gcn0 = (A_out4 * x3[:, :, None, :]).sum(3)
    xm = x_att.mean(-1, keepdims=True)
    h = jnp.matmul(cc1_w[None], xm, precision=_PREC) + cc1_b[:, None]
    h = (h - bn_m[:, None]) * (bn_g / jnp.sqrt(bn_v + BN_EPS))[:, None] \
        + bn_b[:, None]
    h = jax.nn.gelu(h, approximate=False)
    c_att = jax.nn.sigmoid(
        jnp.matmul(cc2_w[None], h, precision=_PREC) + cc2_b[:, None])
    x_gcn = x_gcn0 * c_att
    s_att = jax.nn.sigmoid(
        jnp.matmul(cs_w[None], x_gcn, precision=_PREC) + cs_b[:, None])
    delta = x_gcn + x_att * s_att  # = out - x; x added back on host in fp32
    # int8 quantize per (sample, channel) row; |delta/scale| <= 127 by
    # construction so no clip is needed.
    m = jnp.max(jnp.abs(delta), axis=-1, keepdims=True)
    scale = m * (1.0 / 127.0) + 1e-30
    q = jnp.round(delta / scale).astype(jnp.int8)  # (bs, C_OUT, V)
    return q, scale.astype(jnp.float16)


_PNAMES = ['w1', 'b1', 'w2', 'b2', 'w3', 'b3', 'diff_w', 'diff_b',
           'edge_w', 'edge_b', 'att_w', 'att_b', 'A_static', 'alpha',
           'cc1_w', 'cc1_b', 'bn_g', 'bn_b', 'bn_m', 'bn_v',
           'cc2_w', 'cc2_b', 'cs_w', 'cs_b']

_state = {'param_np': None, 'fused': None, 'mesh': None, 'lib': False,
          'xh_buf': None}

# numpy's fp32->fp16 cast and int8 dequant are scalar on this box (~3ms/call
# combined) despite the CPU having F16C/AVX2; a tiny runtime-compiled helper
# does both at memory bandwidth (~0.5ms). Bit-identical to numpy (RNE). Any
# failure falls back to the numpy path.
_HELPER_C = r'''
#include <immintrin.h>
#include <stddef.h>
void f32_to_f16(const float* src, unsigned short* dst, size_t n) {
    size_t i = 0;
    for (; i + 8 <= n; i += 8) {
        __m256 v = _mm256_loadu_ps(src + i);
        __m128i h = _mm256_cvtps_ph(v, _MM_FROUND_TO_NEAREST_INT);
        _mm_storeu_si128((__m128i*)(dst + i), h);
    }
    for (; i < n; i++) {
        __m128 v = _mm_load_ss(src + i);
        __m128i h = _mm_cvtps_ph(v, _MM_FROUND_TO_NEAREST_INT);
        dst[i] = (unsigned short)_mm_extract_epi16(h, 0);
    }
}
void dequant_add(const signed char* q, const unsigned short* s,
                 const float* x, float* out, size_t rows, size_t v) {
    for (size_t r = 0; r < rows; r++) {
        __m128i hs = _mm_cvtsi32_si128((int)s[r]);
        float scale = _mm_cvtss_f32(_mm_cvtph_ps(hs));
        __m256 vs = _mm256_set1_ps(scale);
        const signed char* qr = q + r * v;
        const float* xr = x + r * v;
        float* outr = out + r * v;
        size_t j = 0;
        for (; j + 8 <= v; j += 8) {
            __m128i qi8 = _mm_loadl_epi64((const __m128i*)(qr + j));
            __m256i qi32 = _mm256_cvtepi8_epi32(qi8);
            __m256 qf = _mm256_cvtepi32_ps(qi32);
            __m256 xv = _mm256_loadu_ps(xr + j);
            _mm256_storeu_ps(outr + j,
                             _mm256_add_ps(_mm256_mul_ps(qf, vs), xv));
        }
        for (; j < v; j++) outr[j] = (float)qr[j] * scale + xr[j];
    }
}
'''


def _get_lib():
    if _state['lib'] is False:
        lib = None
        try:
            d = tempfile.mkdtemp(prefix='cga_helper_')
            src = os.path.join(d, 'helper.c')
            so = os.path.join(d, 'helper.so')
            with open(src, 'w') as f:
                f.write(_HELPER_C)
            for cc in ('cc', 'gcc', 'gcc-11'):
                r = subprocess.run(
                    [cc, '-O3', '-mf16c', '-mavx2', '-shared', '-fPIC',
                     '-o', so, src], capture_output=True)
                if r.returncode == 0:
                    cand = ctypes.CDLL(so)
                    # verify against numpy before trusting it
                    xt = np.random.RandomState(0).randn(1000) \
                        .astype(np.float32)
                    ht = np.empty(1000, np.float16)
                    cand.f32_to_f16(
                        xt.ctypes.data_as(ctypes.c_void_p),
                        ht.ctypes.data_as(ctypes.c_void_p),
                        ctypes.c_size_t(1000))
                    if np.array_equal(ht.view(np.uint16),
                                      xt.astype(np.float16).view(np.uint16)):
                        lib = cand
                    break
        except Exception:
            lib = None
        _state['lib'] = lib
    return _state['lib']


def _get_fused(inputs):
    cur = [np.asarray(inputs[k], dtype=np.float32) for k in _PNAMES]
    old = _state['param_np']
    if (_state['fused'] is None or old is None
            or any(not np.array_equal(a, b) for a, b in zip(cur, old))):
        if _state['mesh'] is None:
            _state['mesh'] = Mesh(np.array(jax.devices()[:N_CORES]), ('b',))
        params = cur
        _state['param_np'] = cur
        _state['fused'] = jax.jit(_shard_map(
            lambda xh: _fused(xh, *params), mesh=_state['mesh'],
            in_specs=P('b'), out_specs=(P('b'), P('b'))))
    return _state['fused']


def kernel(**inputs):
    fused = _get_fused(inputs)
    lib = _get_lib()

    x32 = np.ascontiguousarray(np.asarray(inputs['x'], dtype=np.float32))
    if lib is not None:
        if _state['xh_buf'] is None:
            _state['xh_buf'] = np.empty(x32.shape, np.float16)
        xh = _state['xh_buf']  # safe to reuse: calls are strictly sequential
        lib.f32_to_f16(x32.ctypes.data_as(ctypes.c_void_p),
                       xh.ctypes.data_as(ctypes.c_void_p),
                       ctypes.c_size_t(x32.size))
    else:
        xh = x32.astype(np.float16)

    q_dev, s_dev = fused(xh)
    try:
        q_dev.copy_to_host_async()
        s_dev.copy_to_host_async()
    except Exception:
        pass
    q = np.ascontiguousarray(np.asarray(q_dev))      # (B, C_OUT, V) int8
    scale = np.ascontiguousarray(np.asarray(s_dev))  # (B, C_OUT, 1) fp16
    if lib is not None:
        out = np.empty(x32.shape, np.float32)
        lib.dequant_add(q.ctypes.data_as(ctypes.c_void_p),
                        scale.ctypes.data_as(ctypes.c_void_p),
                        x32.ctypes.data_as(ctypes.c_void_p),
                        out.ctypes.data_as(ctypes.c_void_p),
                        ctypes.c_size_t(B * C_OUT), ctypes.c_size_t(V))
    else:
        out = np.multiply(q, scale.astype(np.float32), casting='unsafe')
        out += x32
    return out

